# revision 4
# baseline (speedup 1.0000x reference)
"""Bass/Trainium2 kernel for nn_DFTLayer: out[b,f,k] = DFT_1024(x[b,f,:]).

reference: real = einsum('bfs,ks->bfk', x, wcos); imag = ... wsin
           out  = complex(real, -imag),  x: [16, 1024, 1024] f32.

Strategy (8 NeuronCores, data-parallel over batch, 2 batches/core):
  - Hermitian symmetry (x real): out[k] = conj(out[N-k]); device covers
    k = 0..255 directly and k = 257..512 via the radix-2 butterfly below;
    col 256 and the k = 513..1023 mirror are host-side.
  - Cosine/sine parity fold (host): u[s] = x[s] + x[N-s], v[s] = x[s] - x[N-s]
    over contraction slots s = 1..512 (u[512] = x[512], v[512] coeff is 0):
        real[k] = x[0] + sum_{s=1..512} u[s] cos(2*pi*k*s/N)
        imag[k] =        sum_{s=1..511} v[s] sin(2*pi*k*s/N)
  - Radix-2 split by parity of s (host): with ue[t] = u[2t+2], uo[t] = u[2t+1]
    (t = 0..255) and likewise ve/vo:
        E[k]  = ue @ wE[:,k],  O[k]  = uo @ wO[:,k]   (cos kernels)
        Es[k] = ve @ wEs[:,k], Os[k] = vo @ wOs[:,k]  (sin kernels)
        real[k]     = x[0] + E[k] + O[k]        k = 0..255
        real[512-k] = x[0] + E[k] - O[k]
        imag[k]     = Es[k] + Os[k],  imag[512-k] = -Es[k] + Os[k]
    This quarters the device matmul work vs the plain folded DFT.
  - Everything crossing HBM is bf16 (inputs pre-folded/transposed/cast on
    host, outputs cast bf16 on the way out): ~8.5 MB per core vs 18 MB for
    the f32 folded version; rel err ~3e-3, well under the 2e-2 gate.
  - Device program: w kernels stationary, moving operand is the transposed
    fold data in 512-wide streams; 64 matmuls, PSUM->SBUF bf16 casts spread
    over ACT/DVE/Pool, inputs on the sync HWDGE queue in consumption order
    (w on the scalar HWDGE queue in parallel), outputs split across the
    gpsimd SWDGE and scalar HWDGE queues. All butterflies/mirrors/
    corrections happen on the host.
"""

import sys

for _p in ("/opt/trn_rl_repo", "/root/.axon_site/_ro/trn_rl_repo"):
    if _p not in sys.path:
        sys.path.append(_p)

import numpy as np
import ml_dtypes
from contextlib import ExitStack

BF16 = np.dtype(ml_dtypes.bfloat16)

N_CORES = 8
B, F_FULL, S = 16, 1024, 1024          # x: [B, F_FULL, S]
F = (B // N_CORES) * F_FULL            # 2048 rows per core
M = 256                                # radix-2 contraction length
KD = 256                               # device freq cols per kernel (k = 0..255)
N_G = F // 512                         # 4 moving-operand groups of 512 rows

_CACHE = {}


def _build():
    """Build + compile the per-core Bass program (cached)."""
    if "nc" in _CACHE:
        return _CACHE["nc"]

    from concourse import bacc, tile, mybir

    f32 = mybir.dt.float32
    bf16 = mybir.dt.bfloat16

    nc = bacc.Bacc("TRN2", target_bir_lowering=False, debug=False)

    # uv row-block b = inp*2 + h (inp in ue,uo,ve,vo; h = row-half); within a
    # block: partition p, cols = tc*1024 + j for t = tc*128 + p, row h*1024+j
    uv_d = nc.dram_tensor("uv", [8 * 128, F], bf16, kind="ExternalInput")
    # w: partition p, cols = tc*1024 + (kern*2 + kc)*128 + q, k = kc*128 + q
    w_d = nc.dram_tensor("w", [128, 2 * 4 * KD], bf16, kind="ExternalInput")
    # eo rows: (kern*2 + kc)*128 + q  (freq k = kc*128 + q), cols: core rows
    eo_d = nc.dram_tensor("eo", [8 * 128, F], bf16, kind="ExternalOutput")

    with tile.TileContext(nc) as tc, ExitStack() as ctx:
        wpool = ctx.enter_context(tc.tile_pool(name="w", bufs=1))
        opool = ctx.enter_context(tc.tile_pool(name="o", bufs=3))
        ppool = ctx.enter_context(tc.tile_pool(name="p", bufs=2, space="PSUM"))

        # stationary DFT kernels on the scalar HWDGE queue, in parallel with
        # the first uv block on sync
        w_t = wpool.tile([128, 2048], bf16, tag="w")
        nc.scalar.dma_start(w_t[:], w_d[:])

        # fold data: 8 blocks [128, 2048], sync queue, consumption order
        uv_ts = []
        for bidx in range(8):
            uv_t = wpool.tile([128, F], bf16, tag=f"uv{bidx}")
            nc.sync.dma_start(uv_t[:], uv_d[bidx * 128:(bidx + 1) * 128, :])
            uv_ts.append(uv_t)

        for kern in range(4):
            for kc in range(2):
                p = kern * 2 + kc
                last = p == 7
                ps = ppool.tile([128, N_G, 512], f32)
                for g in range(N_G):
                    src = uv_ts[kern * 2 + g // 2]
                    c0 = (g % 2) * 512
                    for t in range(2):
                        lhsT = w_t[:, t * 1024 + p * 128:t * 1024 + (p + 1) * 128]
                        nc.tensor.matmul(
                            ps[:, g, :],
                            lhsT,
                            src[:, t * 1024 + c0:t * 1024 + c0 + 512],
                            start=(t == 0),
                            stop=(t == 1),
                        )
                out_t = opool.tile([128, F], bf16)
                if not last:
                    # PSUM readers: only ACT + DVE (gpsimd cannot touch PSUM)
                    nc.scalar.copy(out_t[:, 0:512], ps[:, 0, :])
                    nc.vector.tensor_copy(out_t[:, 512:1024], ps[:, 1, :])
                    nc.scalar.copy(out_t[:, 1024:1536], ps[:, 2, :])
                    nc.vector.tensor_copy(out_t[:, 1536:2048], ps[:, 3, :])
                else:
                    # tail: halve each cast across ACT/DVE so the last
                    # PSUM bank drains in ~350ns
                    for g in range(N_G):
                        a0 = g * 512
                        nc.scalar.copy(out_t[:, a0:a0 + 256], ps[:, g, 0:256])
                        nc.vector.tensor_copy(out_t[:, a0 + 256:a0 + 512],
                                              ps[:, g, 256:512])
                r0 = p * 128
                nc.gpsimd.dma_start(eo_d[r0:r0 + 128, 0:1024], out_t[:, 0:1024])
                if last:
                    nc.scalar.dma_start(eo_d[r0:r0 + 128, 1024:2048],
                                        out_t[:, 1024:2048])
                else:
                    nc.gpsimd.dma_start(eo_d[r0:r0 + 128, 1024:2048],
                                        out_t[:, 1024:2048])

    nc.compile()
    _CACHE["nc"] = nc
    return nc


def kernel(x, wsin, wcos):
    from concourse.bass_utils import run_bass_kernel_spmd

    x = np.asarray(x, dtype=np.float32)
    wsin = np.asarray(wsin, dtype=np.float32)
    wcos = np.asarray(wcos, dtype=np.float32)

    nc = _build()

    # radix-2 DFT kernels, sliced from the provided (symmetric) matrices:
    #   wE[t,k] = cos(2*pi*k*(2t+2)/N), wO[t,k] = cos(2*pi*k*(2t+1)/N)
    wE = wcos[2:513:2, 0:KD]
    wO = wcos[1:512:2, 0:KD]
    wEs = wsin[2:513:2, 0:KD]
    wOs = wsin[1:512:2, 0:KD]
    # [t, kern*256 + c] -> [tc, p, c] -> [p, tc*1024 + c]
    w_np = np.concatenate([wE, wO, wEs, wOs], axis=1).astype(BF16)
    w_np = np.ascontiguousarray(
        w_np.reshape(2, 128, 1024).transpose(1, 0, 2)).reshape(128, 2048)

    # host fold + parity split (f32), then bf16
    xa = x[:, :, 1:512]
    xb = x[:, :, 1023:512:-1]
    u = xa + xb                         # u[s], s = 1..511
    v = xa - xb
    uvp = np.empty((B, F_FULL, 4, M), dtype=np.float32)
    uvp[:, :, 0, :255] = u[:, :, 1::2]  # ue: s = 2,4,..,510
    uvp[:, :, 0, 255] = x[:, :, 512]    # ue[255] <- u[512] = x[512]
    uvp[:, :, 1, :] = u[:, :, 0::2]     # uo: s = 1,3,..,511
    uvp[:, :, 2, :255] = v[:, :, 1::2]  # ve
    uvp[:, :, 2, 255] = 0.0
    uvp[:, :, 3, :] = v[:, :, 0::2]     # vo
    uvp_bf = uvp.astype(BF16)

    bpc = B // N_CORES
    in_maps = []
    for c in range(N_CORES):
        blk = uvp_bf[c * bpc:(c + 1) * bpc].reshape(F, 4, M)
        # [row, i, t] -> [i, t, row] -> [i, tc, p, h, j] -> [i, h, p, tc, j]
        arr = blk.transpose(1, 2, 0).reshape(4, 2, 128, 2, 1024)
        uv_c = np.ascontiguousarray(arr.transpose(0, 3, 2, 1, 4)).reshape(8 * 128, F)
        in_maps.append({"uv": uv_c, "w": w_np})

    res = run_bass_kernel_spmd(
        nc, in_maps, core_ids=list(range(N_CORES)), **_CACHE.get("run_kwargs", {})
    )
    kernel.last_results = res

    # host assembly: butterflies, x[0] correction, col 256, Hermitian mirror
    alt = np.where(np.arange(M) % 2 == 0, np.float32(1.0), np.float32(-1.0))
    out = np.empty((B, F_FULL, S), dtype=np.complex64)
    fv = out.view(np.float32).reshape(B, F_FULL, 2 * S)
    for c in range(N_CORES):
        b0 = c * bpc
        eo = np.asarray(res.results[c]["eo"]).reshape(4, KD, F)
        E = eo[0].T.astype(np.float32)      # [F, KD]
        O = eo[1].T.astype(np.float32)
        Es = eo[2].T.astype(np.float32)
        Os = eo[3].T.astype(np.float32)
        x0 = x[b0:b0 + bpc, :, 0].reshape(F, 1)
        reA = E + O
        reA += x0
        reB = E - O
        reB += x0
        imA = Es + Os
        np.negative(imA, out=imA)           # out.imag = -imag_raw
        imB = Es - Os
        fvb = fv[b0:b0 + bpc].reshape(F, 2 * S)
        fvb[:, 0:2 * KD:2] = reA            # real, k = 0..255
        fvb[:, 1:2 * KD:2] = imA
        fvb[:, 514:1026:2] = reB[:, ::-1]   # real, k = 257..512
        fvb[:, 515:1027:2] = imB[:, ::-1]
        # col 256: even-s cos run is (-1)^(t+1), odd-s sin run is (-1)^t
        ue32 = uvp[b0:b0 + bpc, :, 0, :].reshape(F, M)
        vo32 = uvp[b0:b0 + bpc, :, 3, :].reshape(F, M)
        fvb[:, 512] = x0[:, 0] - ue32 @ alt
        fvb[:, 513] = -(vo32 @ alt)
        # Hermitian mirror: out[k] = conj(out[1024-k]) for k = 513..1023
        fvb[:, 1026::2] = fvb[:, 1022:0:-2]
        fvb[:, 1027::2] = -fvb[:, 1023:1:-2]
    return out


# revision 9
# speedup vs baseline: 1.2374x; 1.2374x over previous
"""Bass/Trainium2 kernel for nn_DFTLayer: out[b,f,k] = DFT_1024(x[b,f,:]).

reference: real = einsum('bfs,ks->bfk', x, wcos); imag = ... wsin
           out  = complex(real, -imag),  x: [16, 1024, 1024] f32.

Strategy (8 NeuronCores, data-parallel over batch, 2 batches/core):
  - Hermitian symmetry (x real): out[k] = conj(out[N-k]); device covers
    k = 0..255 directly and k = 257..512 via the radix-2 butterfly below;
    col 256 and the k = 513..1023 mirror are host-side.
  - Cosine/sine parity fold (host): u[s] = x[s] + x[N-s], v[s] = x[s] - x[N-s]
    over contraction slots s = 1..512 (u[512] = x[512], v[512] coeff is 0):
        real[k] = x[0] + sum_{s=1..512} u[s] cos(2*pi*k*s/N)
        imag[k] =        sum_{s=1..511} v[s] sin(2*pi*k*s/N)
  - Radix-2 split by parity of s (host): with ue[t] = u[2t+2], uo[t] = u[2t+1]
    (t = 0..255) and likewise ve/vo:
        E[k]  = ue @ wE[:,k],  O[k]  = uo @ wO[:,k]   (cos kernels)
        Es[k] = ve @ wEs[:,k], Os[k] = vo @ wOs[:,k]  (sin kernels)
        real[k]     = x[0] + E[k] + O[k]        k = 0..255
        real[512-k] = x[0] + E[k] - O[k]
        imag[k]     = Es[k] + Os[k],  imag[512-k] = -Es[k] + Os[k]
    This quarters the device matmul work vs the plain folded DFT.
  - Everything crossing HBM is bf16 (inputs pre-folded/transposed/cast on
    host, outputs cast bf16 on the way out): ~8.5 MB per core vs 18 MB for
    the f32 folded version; rel err ~3e-3, well under the 2e-2 gate.
  - Device program: w kernels stationary, moving operand is the transposed
    fold data in 512-wide streams; 64 matmuls, PSUM->SBUF bf16 casts spread
    over ACT/DVE/Pool, inputs on the sync HWDGE queue in consumption order
    (w on the scalar HWDGE queue in parallel), outputs split across the
    gpsimd SWDGE and scalar HWDGE queues. All butterflies/mirrors/
    corrections happen on the host.
"""

import sys

for _p in ("/opt/trn_rl_repo", "/root/.axon_site/_ro/trn_rl_repo"):
    if _p not in sys.path:
        sys.path.append(_p)

import numpy as np
import ml_dtypes
from contextlib import ExitStack

BF16 = np.dtype(ml_dtypes.bfloat16)

N_CORES = 8
B, F_FULL, S = 16, 1024, 1024          # x: [B, F_FULL, S]
F = (B // N_CORES) * F_FULL            # 2048 rows per core
M = 256                                # radix-2 contraction length
KD = 256                               # device freq cols per kernel (k = 0..255)
N_G = F // 512                         # 4 moving-operand groups of 512 rows
WARMUP_MM = 8                          # dummy matmuls to ramp the PE p-state

_CACHE = {}


def _build():
    """Build + compile the per-core Bass program (cached)."""
    if "nc" in _CACHE:
        return _CACHE["nc"]

    from concourse import bacc, tile, mybir

    f32 = mybir.dt.float32
    bf16 = mybir.dt.bfloat16

    nc = bacc.Bacc("TRN2", target_bir_lowering=False, debug=False)

    # uv row-block b = inp*2 + h (inp in ue,uo,ve,vo; h = row-half); within a
    # block: partition p, cols = tc*1024 + j for t = tc*128 + p, row h*1024+j
    uv_d = nc.dram_tensor("uv", [8 * 128, F], bf16, kind="ExternalInput")
    # w: partition p, cols = tc*1024 + (kern*2 + kc)*128 + q, k = kc*128 + q
    w_d = nc.dram_tensor("w", [128, 2 * 4 * KD], bf16, kind="ExternalInput")
    # eo rows: (kern*2 + kc)*128 + q  (freq k = kc*128 + q), cols: core rows
    eo_d = nc.dram_tensor("eo", [8 * 128, F], bf16, kind="ExternalOutput")

    rng = np.random.default_rng(7)
    wu_d = nc.inline_tensor(
        rng.standard_normal((128, 128)).astype(np.float32), name="wu")

    with tile.TileContext(nc) as tc, ExitStack() as ctx:
        wpool = ctx.enter_context(tc.tile_pool(name="w", bufs=1))
        opool = ctx.enter_context(tc.tile_pool(name="o", bufs=3))
        ppool = ctx.enter_context(tc.tile_pool(name="p", bufs=3, space="PSUM"))
        wupool = ctx.enter_context(tc.tile_pool(name="wu", bufs=1, space="PSUM"))

        f32r = mybir.dt.float32r

        # stationary DFT kernels on the scalar HWDGE queue (t-halves so the
        # first LDW waits only 256 KB), in parallel with uv block 0 on sync
        w_t = wpool.tile([128, 2, 1024], bf16, tag="w")
        nc.scalar.dma_start(w_t[:, 0, :], w_d[:, 0:1024])
        nc.scalar.dma_start(w_t[:, 1, :], w_d[:, 1024:2048])

        # warm-up operand on the gpsimd SWDGE queue (tiny, lands ~1 us in)
        wu_t = wpool.tile([128, 128], f32, tag="wu")
        nc.gpsimd.dma_start(wu_t[:], wu_d[:])

        # fold data on sync, consumption order; block 0 in t-halves so
        # phase 0's first matmuls start ~1.3 us earlier
        uv_ts = []
        for bidx in range(8):
            uv_t = wpool.tile([128, 2, 1024], bf16, tag=f"uv{bidx}")
            src = uv_d[bidx * 128:(bidx + 1) * 128, :]
            if bidx == 0:
                nc.sync.dma_start(uv_t[:, 0, :], src[:, 0:1024])
                nc.sync.dma_start(uv_t[:, 1, :], src[:, 1024:2048])
            else:
                nc.sync.dma_start(uv_t[:], src.rearrange("p (t j) -> p t j", t=2))
            uv_ts.append(uv_t)

        # p-state warm-up: dummy matmuls ramp the PE clock (0.65 -> 2.4 GHz)
        # while the template boots and the first real operands stream in
        ps_w = wupool.tile([128, 512], f32)
        for i in range(WARMUP_MM):
            nc.tensor.matmul(ps_w[:, 0:128], wu_t[:].bitcast(f32r),
                             wu_t[:].bitcast(f32r), start=True, stop=True)

        for kern in range(4):
            for kc in range(2):
                p = kern * 2 + kc
                last = p == 7
                out_t = opool.tile([128, F], bf16)
                for half in range(2):
                    # half h covers g = 2h, 2h+1 -- both fed by uv block
                    # kern*2 + h; 2-bank PSUM tiles keep WAR hazards fine
                    ps = ppool.tile([128, 2, 512], f32)
                    src = uv_ts[kern * 2 + half]
                    for t in range(2):
                        lhsT = w_t[:, t, p * 128:(p + 1) * 128]
                        for g in range(2):
                            nc.tensor.matmul(
                                ps[:, g, :],
                                lhsT,
                                src[:, t, g * 512:(g + 1) * 512],
                                start=(t == 0),
                                stop=(t == 1),
                            )
                    a0 = half * 1024
                    if not last:
                        # PSUM readers: only ACT + DVE (gpsimd cannot
                        # touch PSUM)
                        nc.scalar.copy(out_t[:, a0:a0 + 512], ps[:, 0, :])
                        nc.vector.tensor_copy(out_t[:, a0 + 512:a0 + 1024],
                                              ps[:, 1, :])
                    else:
                        # tail: halve each cast across ACT/DVE so the last
                        # PSUM bank drains in ~350ns
                        for g in range(2):
                            b0 = a0 + g * 512
                            nc.scalar.copy(out_t[:, b0:b0 + 256],
                                           ps[:, g, 0:256])
                            nc.vector.tensor_copy(out_t[:, b0 + 256:b0 + 512],
                                                  ps[:, g, 256:512])
                # outputs ride the sync HWDGE queue (idle once inputs are in);
                # the last phase splits across sync + gpsimd for the tail
                r0 = p * 128
                if last:
                    nc.sync.dma_start(eo_d[r0:r0 + 128, 0:1024], out_t[:, 0:1024])
                    nc.gpsimd.dma_start(eo_d[r0:r0 + 128, 1024:2048],
                                        out_t[:, 1024:2048])
                else:
                    nc.sync.dma_start(eo_d[r0:r0 + 128, :], out_t[:])

    nc.compile()
    _CACHE["nc"] = nc
    return nc


def kernel(x, wsin, wcos):
    from concourse.bass_utils import run_bass_kernel_spmd

    x = np.asarray(x, dtype=np.float32)
    wsin = np.asarray(wsin, dtype=np.float32)
    wcos = np.asarray(wcos, dtype=np.float32)

    nc = _build()

    # radix-2 DFT kernels, sliced from the provided (symmetric) matrices:
    #   wE[t,k] = cos(2*pi*k*(2t+2)/N), wO[t,k] = cos(2*pi*k*(2t+1)/N)
    wE = wcos[2:513:2, 0:KD]
    wO = wcos[1:512:2, 0:KD]
    wEs = wsin[2:513:2, 0:KD]
    wOs = wsin[1:512:2, 0:KD]
    # [t, kern*256 + c] -> [tc, p, c] -> [p, tc*1024 + c]
    w_np = np.concatenate([wE, wO, wEs, wOs], axis=1).astype(BF16)
    w_np = np.ascontiguousarray(
        w_np.reshape(2, 128, 1024).transpose(1, 0, 2)).reshape(128, 2048)

    # host fold + parity split (f32), then bf16
    xa = x[:, :, 1:512]
    xb = x[:, :, 1023:512:-1]
    u = xa + xb                         # u[s], s = 1..511
    v = xa - xb
    uvp = np.empty((B, F_FULL, 4, M), dtype=np.float32)
    uvp[:, :, 0, :255] = u[:, :, 1::2]  # ue: s = 2,4,..,510
    uvp[:, :, 0, 255] = x[:, :, 512]    # ue[255] <- u[512] = x[512]
    uvp[:, :, 1, :] = u[:, :, 0::2]     # uo: s = 1,3,..,511
    uvp[:, :, 2, :255] = v[:, :, 1::2]  # ve
    uvp[:, :, 2, 255] = 0.0
    uvp[:, :, 3, :] = v[:, :, 0::2]     # vo
    uvp_bf = uvp.astype(BF16)

    bpc = B // N_CORES
    in_maps = []
    for c in range(N_CORES):
        blk = uvp_bf[c * bpc:(c + 1) * bpc].reshape(F, 4, M)
        # [row, i, t] -> [i, t, row] -> [i, tc, p, h, j] -> [i, h, p, tc, j]
        arr = blk.transpose(1, 2, 0).reshape(4, 2, 128, 2, 1024)
        uv_c = np.ascontiguousarray(arr.transpose(0, 3, 2, 1, 4)).reshape(8 * 128, F)
        in_maps.append({"uv": uv_c, "w": w_np})

    res = run_bass_kernel_spmd(
        nc, in_maps, core_ids=list(range(N_CORES)), **_CACHE.get("run_kwargs", {})
    )
    kernel.last_results = res

    # host assembly: butterflies, x[0] correction, col 256, Hermitian mirror
    alt = np.where(np.arange(M) % 2 == 0, np.float32(1.0), np.float32(-1.0))
    out = np.empty((B, F_FULL, S), dtype=np.complex64)
    fv = out.view(np.float32).reshape(B, F_FULL, 2 * S)
    for c in range(N_CORES):
        b0 = c * bpc
        eo = np.asarray(res.results[c]["eo"]).reshape(4, KD, F)
        E = eo[0].T.astype(np.float32)      # [F, KD]
        O = eo[1].T.astype(np.float32)
        Es = eo[2].T.astype(np.float32)
        Os = eo[3].T.astype(np.float32)
        x0 = x[b0:b0 + bpc, :, 0].reshape(F, 1)
        reA = E + O
        reA += x0
        reB = E - O
        reB += x0
        imA = Es + Os
        np.negative(imA, out=imA)           # out.imag = -imag_raw
        imB = Es - Os
        fvb = fv[b0:b0 + bpc].reshape(F, 2 * S)
        fvb[:, 0:2 * KD:2] = reA            # real, k = 0..255
        fvb[:, 1:2 * KD:2] = imA
        fvb[:, 514:1026:2] = reB[:, ::-1]   # real, k = 257..512
        fvb[:, 515:1027:2] = imB[:, ::-1]
        # col 256: even-s cos run is (-1)^(t+1), odd-s sin run is (-1)^t
        ue32 = uvp[b0:b0 + bpc, :, 0, :].reshape(F, M)
        vo32 = uvp[b0:b0 + bpc, :, 3, :].reshape(F, M)
        fvb[:, 512] = x0[:, 0] - ue32 @ alt
        fvb[:, 513] = -(vo32 @ alt)
        # Hermitian mirror: out[k] = conj(out[1024-k]) for k = 513..1023
        fvb[:, 1026::2] = fvb[:, 1022:0:-2]
        fvb[:, 1027::2] = -fvb[:, 1023:1:-2]
    return out


# revision 12
# speedup vs baseline: 1.3400x; 1.0830x over previous
"""Bass/Trainium2 kernel for nn_DFTLayer: out[b,f,k] = DFT_1024(x[b,f,:]).

reference: real = einsum('bfs,ks->bfk', x, wcos); imag = ... wsin
           out  = complex(real, -imag),  x: [16, 1024, 1024] f32.

Strategy (8 NeuronCores, data-parallel over batch, 2 batches/core):
  - Hermitian symmetry (x real): out[k] = conj(out[N-k]); device covers
    k = 0..255 directly and k = 257..512 via the radix-2 butterfly below;
    col 256 and the k = 513..1023 mirror are host-side.
  - Cosine/sine parity fold (host): u[s] = x[s] + x[N-s], v[s] = x[s] - x[N-s]
    over contraction slots s = 1..512 (u[512] = x[512], v[512] coeff is 0):
        real[k] = x[0] + sum_{s=1..512} u[s] cos(2*pi*k*s/N)
        imag[k] =        sum_{s=1..511} v[s] sin(2*pi*k*s/N)
  - Radix-2 split by parity of s (host): with ue[t] = u[2t+2], uo[t] = u[2t+1]
    (t = 0..255) and likewise ve/vo:
        E[k]  = ue @ wE[:,k],  O[k]  = uo @ wO[:,k]   (cos kernels)
        Es[k] = ve @ wEs[:,k], Os[k] = vo @ wOs[:,k]  (sin kernels)
        real[k]     = x[0] + E[k] + O[k]        k = 0..255
        real[512-k] = x[0] + E[k] - O[k]
        imag[k]     = Es[k] + Os[k],  imag[512-k] = -Es[k] + Os[k]
    This quarters the device matmul work vs the plain folded DFT.
  - Everything crossing HBM is bf16 (inputs pre-folded/transposed/cast on
    host, outputs cast bf16 on the way out): ~8.5 MB per core vs 18 MB for
    the f32 folded version; rel err ~3e-3, well under the 2e-2 gate.
  - Device program: w kernels stationary, moving operand is the transposed
    fold data in 512-wide streams; 64 matmuls, PSUM->SBUF bf16 casts spread
    over ACT/DVE/Pool, inputs on the sync HWDGE queue in consumption order
    (w on the scalar HWDGE queue in parallel), outputs split across the
    gpsimd SWDGE and scalar HWDGE queues. All butterflies/mirrors/
    corrections happen on the host.
"""

import sys

for _p in ("/opt/trn_rl_repo", "/root/.axon_site/_ro/trn_rl_repo"):
    if _p not in sys.path:
        sys.path.append(_p)

import numpy as np
import ml_dtypes
from contextlib import ExitStack

BF16 = np.dtype(ml_dtypes.bfloat16)

N_CORES = 8
B, F_FULL, S = 16, 1024, 1024          # x: [B, F_FULL, S]
F = (B // N_CORES) * F_FULL            # 2048 rows per core
M = 256                                # radix-2 contraction length
KD = 256                               # device freq cols per kernel (k = 0..255)
N_G = F // 512                         # 4 moving-operand groups of 512 rows
WARMUP_MM = 8                          # dummy matmuls to ramp the PE p-state

_CACHE = {}


def _build():
    """Build + compile the per-core Bass program (cached)."""
    if "nc" in _CACHE:
        return _CACHE["nc"]

    from concourse import bacc, tile, mybir

    f32 = mybir.dt.float32
    bf16 = mybir.dt.bfloat16

    nc = bacc.Bacc("TRN2", target_bir_lowering=False, debug=False)

    # uv row-block b = inp*2 + h (inp in ue,uo,ve,vo; h = row-half); within a
    # block: partition p, cols = tc*1024 + j for t = tc*128 + p, row h*1024+j
    uv_d = nc.dram_tensor("uv", [8 * 128, F], bf16, kind="ExternalInput")
    # w: partition p, cols = tc*1024 + (kern*2 + kc)*128 + q, k = kc*128 + q
    w_d = nc.dram_tensor("w", [128, 2 * 4 * KD], bf16, kind="ExternalInput")
    # eo rows: (kern*2 + kc)*128 + q  (freq k = kc*128 + q); col blocks
    # [half, g, 512] flatten to the 2048 core rows
    eo_d = nc.dram_tensor("eo", [8 * 128, 2, 2, 512], bf16, kind="ExternalOutput")

    rng = np.random.default_rng(7)
    wu_d = nc.inline_tensor(
        rng.standard_normal((128, 128)).astype(np.float32), name="wu")

    with tile.TileContext(nc) as tc, ExitStack() as ctx:
        wpool = ctx.enter_context(tc.tile_pool(name="w", bufs=1))
        opool = ctx.enter_context(tc.tile_pool(name="o", bufs=3))
        ppool = ctx.enter_context(tc.tile_pool(name="p", bufs=3, space="PSUM"))
        wupool = ctx.enter_context(tc.tile_pool(name="wu", bufs=1, space="PSUM"))

        f32r = mybir.dt.float32r

        # warm-up operand first on the scalar HWDGE queue (tiny, lands fast;
        # the gpsimd SWDGE path adds ~3 us of latency)
        wu_t = wpool.tile([128, 128], f32, tag="wu")
        nc.scalar.dma_start(wu_t[:], wu_d[:])

        # stationary DFT kernels next on scalar (t-halves so the first LDW
        # waits only 256 KB), in parallel with uv block 0 on sync
        w_t = wpool.tile([128, 2, 1024], bf16, tag="w")
        nc.scalar.dma_start(w_t[:, 0, :], w_d[:, 0:1024])
        nc.scalar.dma_start(w_t[:, 1, :], w_d[:, 1024:2048])

        # fold data on sync, consumption order; block 0 in t-halves so
        # phase 0's first matmuls start ~1.3 us earlier
        uv_ts = []
        for bidx in range(8):
            uv_t = wpool.tile([128, 2, 1024], bf16, tag=f"uv{bidx}")
            src = uv_d[bidx * 128:(bidx + 1) * 128, :]
            if bidx == 0:
                nc.sync.dma_start(uv_t[:, 0, :], src[:, 0:1024])
                nc.sync.dma_start(uv_t[:, 1, :], src[:, 1024:2048])
            else:
                nc.sync.dma_start(uv_t[:], src.rearrange("p (t j) -> p t j", t=2))
            uv_ts.append(uv_t)

        # p-state warm-up: dummy matmuls ramp the PE clock (0.65 -> 2.4 GHz)
        # while the template boots and the first real operands stream in
        ps_w = wupool.tile([128, 512], f32)
        for i in range(WARMUP_MM):
            nc.tensor.matmul(ps_w[:, 0:128], wu_t[:].bitcast(f32r),
                             wu_t[:].bitcast(f32r), start=True, stop=True)

        for kern in range(4):
            for kc in range(2):
                p = kern * 2 + kc
                last = p == 7
                out_t = opool.tile([128, 2, 2, 512], bf16)
                r0 = p * 128
                for half in range(2):
                    # half h covers g = 2h, 2h+1 -- both fed by uv block
                    # kern*2 + h; 2-bank PSUM tiles keep WAR hazards fine
                    ps = ppool.tile([128, 2, 512], f32)
                    src = uv_ts[kern * 2 + half]
                    for t in range(2):
                        lhsT = w_t[:, t, p * 128:(p + 1) * 128]
                        for g in range(2):
                            nc.tensor.matmul(
                                ps[:, g, :],
                                lhsT,
                                src[:, t, g * 512:(g + 1) * 512],
                                start=(t == 0),
                                stop=(t == 1),
                            )
                    # one combined 2-bank cast per half, alternating engines
                    # (cuts per-op overhead; only ACT/DVE can read PSUM)
                    if not last:
                        if (p * 2 + half) % 2 == 0:
                            nc.scalar.copy(out_t[:, half], ps[:])
                        else:
                            nc.vector.tensor_copy(out_t[:, half], ps[:])
                        nc.sync.dma_start(eo_d[r0:r0 + 128, half], out_t[:, half])
                    else:
                        # tail: split across both engines so the last PSUM
                        # banks drain in ~700ns; second desc on scalar
                        nc.scalar.copy(out_t[:, half, 0], ps[:, 0, :])
                        nc.vector.tensor_copy(out_t[:, half, 1], ps[:, 1, :])
                        eng = nc.sync if half == 0 else nc.scalar
                        eng.dma_start(eo_d[r0:r0 + 128, half], out_t[:, half])

    nc.compile()
    _CACHE["nc"] = nc
    return nc


def kernel(x, wsin, wcos):
    from concourse.bass_utils import run_bass_kernel_spmd

    x = np.asarray(x, dtype=np.float32)
    wsin = np.asarray(wsin, dtype=np.float32)
    wcos = np.asarray(wcos, dtype=np.float32)

    nc = _build()

    # radix-2 DFT kernels, sliced from the provided (symmetric) matrices:
    #   wE[t,k] = cos(2*pi*k*(2t+2)/N), wO[t,k] = cos(2*pi*k*(2t+1)/N)
    wE = wcos[2:513:2, 0:KD]
    wO = wcos[1:512:2, 0:KD]
    wEs = wsin[2:513:2, 0:KD]
    wOs = wsin[1:512:2, 0:KD]
    # [t, kern*256 + c] -> [tc, p, c] -> [p, tc*1024 + c]
    w_np = np.concatenate([wE, wO, wEs, wOs], axis=1).astype(BF16)
    w_np = np.ascontiguousarray(
        w_np.reshape(2, 128, 1024).transpose(1, 0, 2)).reshape(128, 2048)

    # host fold + parity split (f32), then bf16
    xa = x[:, :, 1:512]
    xb = x[:, :, 1023:512:-1]
    u = xa + xb                         # u[s], s = 1..511
    v = xa - xb
    uvp = np.empty((B, F_FULL, 4, M), dtype=np.float32)
    uvp[:, :, 0, :255] = u[:, :, 1::2]  # ue: s = 2,4,..,510
    uvp[:, :, 0, 255] = x[:, :, 512]    # ue[255] <- u[512] = x[512]
    uvp[:, :, 1, :] = u[:, :, 0::2]     # uo: s = 1,3,..,511
    uvp[:, :, 2, :255] = v[:, :, 1::2]  # ve
    uvp[:, :, 2, 255] = 0.0
    uvp[:, :, 3, :] = v[:, :, 0::2]     # vo
    uvp_bf = uvp.astype(BF16)

    bpc = B // N_CORES
    in_maps = []
    for c in range(N_CORES):
        blk = uvp_bf[c * bpc:(c + 1) * bpc].reshape(F, 4, M)
        # [row, i, t] -> [i, t, row] -> [i, tc, p, h, j] -> [i, h, p, tc, j]
        arr = blk.transpose(1, 2, 0).reshape(4, 2, 128, 2, 1024)
        uv_c = np.ascontiguousarray(arr.transpose(0, 3, 2, 1, 4)).reshape(8 * 128, F)
        in_maps.append({"uv": uv_c, "w": w_np})

    res = run_bass_kernel_spmd(
        nc, in_maps, core_ids=list(range(N_CORES)), **_CACHE.get("run_kwargs", {})
    )
    kernel.last_results = res

    # host assembly: butterflies, x[0] correction, col 256, Hermitian mirror
    alt = np.where(np.arange(M) % 2 == 0, np.float32(1.0), np.float32(-1.0))
    out = np.empty((B, F_FULL, S), dtype=np.complex64)
    fv = out.view(np.float32).reshape(B, F_FULL, 2 * S)
    for c in range(N_CORES):
        b0 = c * bpc
        eo = np.asarray(res.results[c]["eo"]).reshape(4, KD, F)
        E = eo[0].T.astype(np.float32)      # [F, KD]
        O = eo[1].T.astype(np.float32)
        Es = eo[2].T.astype(np.float32)
        Os = eo[3].T.astype(np.float32)
        x0 = x[b0:b0 + bpc, :, 0].reshape(F, 1)
        reA = E + O
        reA += x0
        reB = E - O
        reB += x0
        imA = Es + Os
        np.negative(imA, out=imA)           # out.imag = -imag_raw
        imB = Es - Os
        fvb = fv[b0:b0 + bpc].reshape(F, 2 * S)
        fvb[:, 0:2 * KD:2] = reA            # real, k = 0..255
        fvb[:, 1:2 * KD:2] = imA
        fvb[:, 514:1026:2] = reB[:, ::-1]   # real, k = 257..512
        fvb[:, 515:1027:2] = imB[:, ::-1]
        # col 256: even-s cos run is (-1)^(t+1), odd-s sin run is (-1)^t
        ue32 = uvp[b0:b0 + bpc, :, 0, :].reshape(F, M)
        vo32 = uvp[b0:b0 + bpc, :, 3, :].reshape(F, M)
        fvb[:, 512] = x0[:, 0] - ue32 @ alt
        fvb[:, 513] = -(vo32 @ alt)
        # Hermitian mirror: out[k] = conj(out[1024-k]) for k = 513..1023
        fvb[:, 1026::2] = fvb[:, 1022:0:-2]
        fvb[:, 1027::2] = -fvb[:, 1023:1:-2]
    return out


# revision 15
# speedup vs baseline: 1.3672x; 1.0203x over previous
"""Bass/Trainium2 kernel for nn_DFTLayer: out[b,f,k] = DFT_1024(x[b,f,:]).

reference: real = einsum('bfs,ks->bfk', x, wcos); imag = ... wsin
           out  = complex(real, -imag),  x: [16, 1024, 1024] f32.

Strategy (8 NeuronCores, data-parallel over batch, 2 batches/core):
  - Hermitian symmetry (x real): out[k] = conj(out[N-k]); device covers
    k = 0..255 directly and k = 257..512 via the radix-2 butterfly below;
    col 256 and the k = 513..1023 mirror are host-side.
  - Cosine/sine parity fold (host): u[s] = x[s] + x[N-s], v[s] = x[s] - x[N-s]
    over contraction slots s = 1..512 (u[512] = x[512], v[512] coeff is 0):
        real[k] = x[0] + sum_{s=1..512} u[s] cos(2*pi*k*s/N)
        imag[k] =        sum_{s=1..511} v[s] sin(2*pi*k*s/N)
  - Radix-2 split by parity of s (host): with ue[t] = u[2t+2], uo[t] = u[2t+1]
    (t = 0..255) and likewise ve/vo:
        E[k]  = ue @ wE[:,k],  O[k]  = uo @ wO[:,k]   (cos kernels)
        Es[k] = ve @ wEs[:,k], Os[k] = vo @ wOs[:,k]  (sin kernels)
        real[k]     = x[0] + E[k] + O[k]        k = 0..255
        real[512-k] = x[0] + E[k] - O[k]
        imag[k]     = Es[k] + Os[k],  imag[512-k] = -Es[k] + Os[k]
    This quarters the device matmul work vs the plain folded DFT.
  - Everything crossing HBM is bf16 (inputs pre-folded/transposed/cast on
    host, outputs cast bf16 on the way out): ~8.5 MB per core vs 18 MB for
    the f32 folded version; rel err ~3e-3, well under the 2e-2 gate.
  - Device program: w kernels stationary, moving operand is the transposed
    fold data in 512-wide streams; 64 matmuls, PSUM->SBUF bf16 casts spread
    over ACT/DVE/Pool, inputs on the sync HWDGE queue in consumption order
    (w on the scalar HWDGE queue in parallel), outputs split across the
    gpsimd SWDGE and scalar HWDGE queues. All butterflies/mirrors/
    corrections happen on the host.
"""

import sys

for _p in ("/opt/trn_rl_repo", "/root/.axon_site/_ro/trn_rl_repo"):
    if _p not in sys.path:
        sys.path.append(_p)

import numpy as np
import ml_dtypes
from contextlib import ExitStack

BF16 = np.dtype(ml_dtypes.bfloat16)

N_CORES = 8
B, F_FULL, S = 16, 1024, 1024          # x: [B, F_FULL, S]
F = (B // N_CORES) * F_FULL            # 2048 rows per core
M = 256                                # radix-2 contraction length
KD = 256                               # device freq cols per kernel (k = 0..255)
N_G = F // 512                         # 4 moving-operand groups of 512 rows
WARMUP_MM = 12                         # dummy matmuls to ramp the PE p-state

_CACHE = {}


def _build():
    """Build + compile the per-core Bass program (cached)."""
    if "nc" in _CACHE:
        return _CACHE["nc"]

    from concourse import bacc, tile, mybir

    f32 = mybir.dt.float32
    bf16 = mybir.dt.bfloat16

    nc = bacc.Bacc("TRN2", target_bir_lowering=False, debug=False)

    # uv row-block b = inp*2 + h (inp in ue,uo,ve,vo; h = row-half); within a
    # block: partition p, cols = tc*1024 + j for t = tc*128 + p, row h*1024+j
    uv_d = nc.dram_tensor("uv", [8 * 128, F], bf16, kind="ExternalInput")
    # w: partition p, cols = tc*1024 + (kern*2 + kc)*128 + q, k = kc*128 + q
    w_d = nc.dram_tensor("w", [128, 2 * 4 * KD], bf16, kind="ExternalInput")
    # eo rows: (kern*2 + kc)*128 + q  (freq k = kc*128 + q); col blocks
    # [half, g, 512] flatten to the 2048 core rows
    eo_d = nc.dram_tensor("eo", [8 * 128, 2, 2, 512], bf16, kind="ExternalOutput")

    with tile.TileContext(nc) as tc, ExitStack() as ctx:
        wpool = ctx.enter_context(tc.tile_pool(name="w", bufs=1))
        opool = ctx.enter_context(tc.tile_pool(name="o", bufs=3))
        ppool = ctx.enter_context(tc.tile_pool(name="p", bufs=3, space="PSUM"))
        wupool = ctx.enter_context(tc.tile_pool(name="wu", bufs=1, space="PSUM"))

        f32r = mybir.dt.float32r

        # warm-up operand needs no DMA at all: a gpsimd memset right after
        # the template prologue (partial-partition-row DMAs proved ~4x
        # slower per byte than full 4 KB rows, so keep them off this path)
        wu_t = wpool.tile([128, 128], f32, tag="wu")
        nc.gpsimd.memset(wu_t[:], 1.0)

        # stationary DFT kernels: one full-row desc on the scalar HWDGE queue
        w_t = wpool.tile([128, 2, 1024], bf16, tag="w")
        nc.scalar.dma_start(w_t[:], w_d[:].rearrange("p (t j) -> p t j", t=2))

        # fold data on sync, consumption order, whole 4 KB-row blocks
        uv_ts = []
        for bidx in range(8):
            uv_t = wpool.tile([128, 2, 1024], bf16, tag=f"uv{bidx}")
            src = uv_d[bidx * 128:(bidx + 1) * 128, :]
            nc.sync.dma_start(uv_t[:], src.rearrange("p (t j) -> p t j", t=2))
            uv_ts.append(uv_t)

        # p-state warm-up: dummy matmuls ramp the PE clock (0.65 -> 2.4 GHz)
        # while the template boots and the first real operands stream in
        ps_w = wupool.tile([128, 512], f32)
        for i in range(WARMUP_MM):
            nc.tensor.matmul(ps_w[:, 0:128], wu_t[:].bitcast(f32r),
                             wu_t[:].bitcast(f32r), start=True, stop=True)

        for kern in range(4):
            for kc in range(2):
                p = kern * 2 + kc
                last = p == 7
                out_t = opool.tile([128, 2, 2, 512], bf16)
                r0 = p * 128
                for half in range(2):
                    # half h covers g = 2h, 2h+1 -- both fed by uv block
                    # kern*2 + h; 2-bank PSUM tiles keep WAR hazards fine
                    ps = ppool.tile([128, 2, 512], f32)
                    src = uv_ts[kern * 2 + half]
                    for t in range(2):
                        lhsT = w_t[:, t, p * 128:(p + 1) * 128]
                        for g in range(2):
                            nc.tensor.matmul(
                                ps[:, g, :],
                                lhsT,
                                src[:, t, g * 512:(g + 1) * 512],
                                start=(t == 0),
                                stop=(t == 1),
                            )
                    # one combined 2-bank cast per half, alternating engines
                    # (cuts per-op overhead; only ACT/DVE can read PSUM)
                    if not last:
                        if (p * 2 + half) % 2 == 0:
                            nc.scalar.copy(out_t[:, half], ps[:])
                        else:
                            nc.vector.tensor_copy(out_t[:, half], ps[:])
                        nc.sync.dma_start(eo_d[r0:r0 + 128, half], out_t[:, half])
                    else:
                        # tail: split across both engines so the last PSUM
                        # banks drain in ~700ns; second desc on scalar
                        nc.scalar.copy(out_t[:, half, 0], ps[:, 0, :])
                        nc.vector.tensor_copy(out_t[:, half, 1], ps[:, 1, :])
                        eng = nc.sync if half == 0 else nc.scalar
                        eng.dma_start(eo_d[r0:r0 + 128, half], out_t[:, half])

    nc.compile()
    _CACHE["nc"] = nc
    return nc


def kernel(x, wsin, wcos):
    from concourse.bass_utils import run_bass_kernel_spmd

    x = np.asarray(x, dtype=np.float32)
    wsin = np.asarray(wsin, dtype=np.float32)
    wcos = np.asarray(wcos, dtype=np.float32)

    nc = _build()

    # radix-2 DFT kernels, sliced from the provided (symmetric) matrices:
    #   wE[t,k] = cos(2*pi*k*(2t+2)/N), wO[t,k] = cos(2*pi*k*(2t+1)/N)
    wE = wcos[2:513:2, 0:KD]
    wO = wcos[1:512:2, 0:KD]
    wEs = wsin[2:513:2, 0:KD]
    wOs = wsin[1:512:2, 0:KD]
    # [t, kern*256 + c] -> [tc, p, c] -> [p, tc*1024 + c]
    w_np = np.concatenate([wE, wO, wEs, wOs], axis=1).astype(BF16)
    w_np = np.ascontiguousarray(
        w_np.reshape(2, 128, 1024).transpose(1, 0, 2)).reshape(128, 2048)

    # host fold + parity split (f32), then bf16
    xa = x[:, :, 1:512]
    xb = x[:, :, 1023:512:-1]
    u = xa + xb                         # u[s], s = 1..511
    v = xa - xb
    uvp = np.empty((B, F_FULL, 4, M), dtype=np.float32)
    uvp[:, :, 0, :255] = u[:, :, 1::2]  # ue: s = 2,4,..,510
    uvp[:, :, 0, 255] = x[:, :, 512]    # ue[255] <- u[512] = x[512]
    uvp[:, :, 1, :] = u[:, :, 0::2]     # uo: s = 1,3,..,511
    uvp[:, :, 2, :255] = v[:, :, 1::2]  # ve
    uvp[:, :, 2, 255] = 0.0
    uvp[:, :, 3, :] = v[:, :, 0::2]     # vo
    uvp_bf = uvp.astype(BF16)

    bpc = B // N_CORES
    in_maps = []
    for c in range(N_CORES):
        blk = uvp_bf[c * bpc:(c + 1) * bpc].reshape(F, 4, M)
        # [row, i, t] -> [i, t, row] -> [i, tc, p, h, j] -> [i, h, p, tc, j]
        arr = blk.transpose(1, 2, 0).reshape(4, 2, 128, 2, 1024)
        uv_c = np.ascontiguousarray(arr.transpose(0, 3, 2, 1, 4)).reshape(8 * 128, F)
        in_maps.append({"uv": uv_c, "w": w_np})

    res = run_bass_kernel_spmd(
        nc, in_maps, core_ids=list(range(N_CORES)), **_CACHE.get("run_kwargs", {})
    )
    kernel.last_results = res

    # host assembly: butterflies, x[0] correction, col 256, Hermitian mirror
    alt = np.where(np.arange(M) % 2 == 0, np.float32(1.0), np.float32(-1.0))
    out = np.empty((B, F_FULL, S), dtype=np.complex64)
    fv = out.view(np.float32).reshape(B, F_FULL, 2 * S)
    for c in range(N_CORES):
        b0 = c * bpc
        eo = np.asarray(res.results[c]["eo"]).reshape(4, KD, F)
        E = eo[0].T.astype(np.float32)      # [F, KD]
        O = eo[1].T.astype(np.float32)
        Es = eo[2].T.astype(np.float32)
        Os = eo[3].T.astype(np.float32)
        x0 = x[b0:b0 + bpc, :, 0].reshape(F, 1)
        reA = E + O
        reA += x0
        reB = E - O
        reB += x0
        imA = Es + Os
        np.negative(imA, out=imA)           # out.imag = -imag_raw
        imB = Es - Os
        fvb = fv[b0:b0 + bpc].reshape(F, 2 * S)
        fvb[:, 0:2 * KD:2] = reA            # real, k = 0..255
        fvb[:, 1:2 * KD:2] = imA
        fvb[:, 514:1026:2] = reB[:, ::-1]   # real, k = 257..512
        fvb[:, 515:1027:2] = imB[:, ::-1]
        # col 256: even-s cos run is (-1)^(t+1), odd-s sin run is (-1)^t
        ue32 = uvp[b0:b0 + bpc, :, 0, :].reshape(F, M)
        vo32 = uvp[b0:b0 + bpc, :, 3, :].reshape(F, M)
        fvb[:, 512] = x0[:, 0] - ue32 @ alt
        fvb[:, 513] = -(vo32 @ alt)
        # Hermitian mirror: out[k] = conj(out[1024-k]) for k = 513..1023
        fvb[:, 1026::2] = fvb[:, 1022:0:-2]
        fvb[:, 1027::2] = -fvb[:, 1023:1:-2]
    return out


# revision 18
# speedup vs baseline: 1.4540x; 1.0635x over previous
"""Bass/Trainium2 kernel for nn_DFTLayer: out[b,f,k] = DFT_1024(x[b,f,:]).

reference: real = einsum('bfs,ks->bfk', x, wcos); imag = ... wsin
           out  = complex(real, -imag),  x: [16, 1024, 1024] f32.

Strategy (8 NeuronCores, data-parallel over batch, 2 batches/core):
  - Hermitian symmetry (x real): out[k] = conj(out[N-k]); device covers
    k = 0..255 directly and k = 257..512 via the radix-2 butterfly below;
    col 256 and the k = 513..1023 mirror are host-side.
  - Cosine/sine parity fold (host): u[s] = x[s] + x[N-s], v[s] = x[s] - x[N-s]
    over contraction slots s = 1..512 (u[512] = x[512], v[512] coeff is 0):
        real[k] = x[0] + sum_{s=1..512} u[s] cos(2*pi*k*s/N)
        imag[k] =        sum_{s=1..511} v[s] sin(2*pi*k*s/N)
  - Radix-2 split by parity of s (host): with ue[t] = u[2t+2], uo[t] = u[2t+1]
    (t = 0..255) and likewise ve/vo:
        E[k]  = ue @ wE[:,k],  O[k]  = uo @ wO[:,k]   (cos kernels)
        Es[k] = ve @ wEs[:,k], Os[k] = vo @ wOs[:,k]  (sin kernels)
        real[k]     = x[0] + E[k] + O[k]        k = 0..255
        real[512-k] = x[0] + E[k] - O[k]
        imag[k]     = Es[k] + Os[k],  imag[512-k] = -Es[k] + Os[k]
    This quarters the device matmul work vs the plain folded DFT.
  - Everything crossing HBM is bf16 (inputs pre-folded/transposed/cast on
    host, outputs cast bf16 on the way out): ~8.5 MB per core vs 18 MB for
    the f32 folded version; rel err ~3e-3, well under the 2e-2 gate.
  - Device program: w kernels stationary, moving operand is the transposed
    fold data in 512-wide streams; 64 matmuls, PSUM->SBUF bf16 casts spread
    over ACT/DVE/Pool, inputs on the sync HWDGE queue in consumption order
    (w on the scalar HWDGE queue in parallel), outputs split across the
    gpsimd SWDGE and scalar HWDGE queues. All butterflies/mirrors/
    corrections happen on the host.
"""

import sys

for _p in ("/opt/trn_rl_repo", "/root/.axon_site/_ro/trn_rl_repo"):
    if _p not in sys.path:
        sys.path.append(_p)

import numpy as np
import ml_dtypes
from contextlib import ExitStack

BF16 = np.dtype(ml_dtypes.bfloat16)

N_CORES = 8
B, F_FULL, S = 16, 1024, 1024          # x: [B, F_FULL, S]
F = (B // N_CORES) * F_FULL            # 2048 rows per core
M = 256                                # radix-2 contraction length
KD = 256                               # device freq cols per kernel (k = 0..255)
N_G = F // 512                         # 4 moving-operand groups of 512 rows
WARMUP_MM = 12                         # dummy matmuls to ramp the PE p-state

_CACHE = {}


def _build():
    """Build + compile the per-core Bass program (cached)."""
    if "nc" in _CACHE:
        return _CACHE["nc"]

    from concourse import bacc, tile, mybir

    f32 = mybir.dt.float32
    bf16 = mybir.dt.bfloat16

    nc = bacc.Bacc("TRN2", target_bir_lowering=False, debug=False)

    # uv row-block b = inp*2 + h (inp in ue,uo,ve,vo; h = row-half); within a
    # block: partition p, cols = tc*1024 + j for t = tc*128 + p, row h*1024+j
    uv_d = nc.dram_tensor("uv", [8 * 128, F], bf16, kind="ExternalInput")
    # w: partition p, cols = tc*1024 + (kern*2 + kc)*128 + q, k = kc*128 + q
    w_d = nc.dram_tensor("w", [128, 2 * 4 * KD], bf16, kind="ExternalInput")
    # eo rows: (kern*2 + kc)*128 + q  (freq k = kc*128 + q); col blocks
    # [half, g, 512] flatten to the 2048 core rows
    eo_d = nc.dram_tensor("eo", [8 * 128, 2, 2, 512], bf16, kind="ExternalOutput")

    with tile.TileContext(nc) as tc, ExitStack() as ctx:
        wpool = ctx.enter_context(tc.tile_pool(name="w", bufs=1))
        opool = ctx.enter_context(tc.tile_pool(name="o", bufs=3))
        ppool = ctx.enter_context(tc.tile_pool(name="p", bufs=4, space="PSUM"))

        f32r = mybir.dt.float32r

        # warm-up operand needs no DMA at all: a gpsimd memset right after
        # the template prologue (partial-partition-row DMAs proved ~4x
        # slower per byte than full 4 KB rows, so keep them off this path)
        wu_t = wpool.tile([128, 128], f32, tag="wu")
        nc.gpsimd.memset(wu_t[:], 1.0)

        # stationary DFT kernels: one full-row desc on the scalar HWDGE queue
        w_t = wpool.tile([128, 2, 1024], bf16, tag="w")
        nc.scalar.dma_start(w_t[:], w_d[:].rearrange("p (t j) -> p t j", t=2))

        # fold data on sync, consumption order, whole 4 KB-row blocks
        uv_ts = []
        for bidx in range(8):
            uv_t = wpool.tile([128, 2, 1024], bf16, tag=f"uv{bidx}")
            src = uv_d[bidx * 128:(bidx + 1) * 128, :]
            nc.sync.dma_start(uv_t[:], src.rearrange("p (t j) -> p t j", t=2))
            uv_ts.append(uv_t)

        # p-state warm-up: dummy matmuls ramp the PE clock (0.65 -> 2.4 GHz)
        # while the template boots and the first real operands stream in.
        # The tile comes from the main PSUM ring; its slot recycles after
        # the strictly-later real matmuls of the first reuse.
        ps_w = ppool.tile([128, 2, 512], f32, tag="ps")
        for i in range(WARMUP_MM):
            nc.tensor.matmul(ps_w[:, i % 2, 0:128], wu_t[:].bitcast(f32r),
                             wu_t[:].bitcast(f32r), start=True, stop=True)

        for kern in range(4):
            for kc in range(2):
                p = kern * 2 + kc
                last = p == 7
                out_t = opool.tile([128, 2, 2, 512], bf16)
                r0 = p * 128
                for half in range(2):
                    # half h covers g = 2h, 2h+1 -- both fed by uv block
                    # kern*2 + h; 2-bank PSUM tiles keep WAR hazards fine
                    ps = ppool.tile([128, 2, 512], f32, tag="ps")
                    src = uv_ts[kern * 2 + half]
                    for t in range(2):
                        lhsT = w_t[:, t, p * 128:(p + 1) * 128]
                        for g in range(2):
                            nc.tensor.matmul(
                                ps[:, g, :],
                                lhsT,
                                src[:, t, g * 512:(g + 1) * 512],
                                start=(t == 0),
                                stop=(t == 1),
                            )
                    # one combined 2-bank cast per half, alternating engines
                    # (cuts per-op overhead; only ACT/DVE can read PSUM)
                    if not last:
                        if (p * 2 + half) % 2 == 0:
                            nc.scalar.copy(out_t[:, half], ps[:])
                        else:
                            nc.vector.tensor_copy(out_t[:, half], ps[:])
                        nc.sync.dma_start(eo_d[r0:r0 + 128, half], out_t[:, half])
                    else:
                        # tail: split across both engines so the last PSUM
                        # banks drain in ~700ns; second desc on scalar
                        nc.scalar.copy(out_t[:, half, 0], ps[:, 0, :])
                        nc.vector.tensor_copy(out_t[:, half, 1], ps[:, 1, :])
                        eng = nc.sync if half == 0 else nc.scalar
                        eng.dma_start(eo_d[r0:r0 + 128, half], out_t[:, half])

    nc.compile()
    _CACHE["nc"] = nc
    return nc


def kernel(x, wsin, wcos):
    from concourse.bass_utils import run_bass_kernel_spmd

    x = np.asarray(x, dtype=np.float32)
    wsin = np.asarray(wsin, dtype=np.float32)
    wcos = np.asarray(wcos, dtype=np.float32)

    nc = _build()

    # radix-2 DFT kernels, sliced from the provided (symmetric) matrices:
    #   wE[t,k] = cos(2*pi*k*(2t+2)/N), wO[t,k] = cos(2*pi*k*(2t+1)/N)
    wE = wcos[2:513:2, 0:KD]
    wO = wcos[1:512:2, 0:KD]
    wEs = wsin[2:513:2, 0:KD]
    wOs = wsin[1:512:2, 0:KD]
    # [t, kern*256 + c] -> [tc, p, c] -> [p, tc*1024 + c]
    w_np = np.concatenate([wE, wO, wEs, wOs], axis=1).astype(BF16)
    w_np = np.ascontiguousarray(
        w_np.reshape(2, 128, 1024).transpose(1, 0, 2)).reshape(128, 2048)

    # host fold + parity split (f32), then bf16
    xa = x[:, :, 1:512]
    xb = x[:, :, 1023:512:-1]
    u = xa + xb                         # u[s], s = 1..511
    v = xa - xb
    uvp = np.empty((B, F_FULL, 4, M), dtype=np.float32)
    uvp[:, :, 0, :255] = u[:, :, 1::2]  # ue: s = 2,4,..,510
    uvp[:, :, 0, 255] = x[:, :, 512]    # ue[255] <- u[512] = x[512]
    uvp[:, :, 1, :] = u[:, :, 0::2]     # uo: s = 1,3,..,511
    uvp[:, :, 2, :255] = v[:, :, 1::2]  # ve
    uvp[:, :, 2, 255] = 0.0
    uvp[:, :, 3, :] = v[:, :, 0::2]     # vo
    uvp_bf = uvp.astype(BF16)

    bpc = B // N_CORES
    in_maps = []
    for c in range(N_CORES):
        blk = uvp_bf[c * bpc:(c + 1) * bpc].reshape(F, 4, M)
        # [row, i, t] -> [i, t, row] -> [i, tc, p, h, j] -> [i, h, p, tc, j]
        arr = blk.transpose(1, 2, 0).reshape(4, 2, 128, 2, 1024)
        uv_c = np.ascontiguousarray(arr.transpose(0, 3, 2, 1, 4)).reshape(8 * 128, F)
        in_maps.append({"uv": uv_c, "w": w_np})

    res = run_bass_kernel_spmd(
        nc, in_maps, core_ids=list(range(N_CORES)), **_CACHE.get("run_kwargs", {})
    )
    kernel.last_results = res

    # host assembly: butterflies, x[0] correction, col 256, Hermitian mirror
    alt = np.where(np.arange(M) % 2 == 0, np.float32(1.0), np.float32(-1.0))
    out = np.empty((B, F_FULL, S), dtype=np.complex64)
    fv = out.view(np.float32).reshape(B, F_FULL, 2 * S)
    for c in range(N_CORES):
        b0 = c * bpc
        eo = np.asarray(res.results[c]["eo"]).reshape(4, KD, F)
        E = eo[0].T.astype(np.float32)      # [F, KD]
        O = eo[1].T.astype(np.float32)
        Es = eo[2].T.astype(np.float32)
        Os = eo[3].T.astype(np.float32)
        x0 = x[b0:b0 + bpc, :, 0].reshape(F, 1)
        reA = E + O
        reA += x0
        reB = E - O
        reB += x0
        imA = Es + Os
        np.negative(imA, out=imA)           # out.imag = -imag_raw
        imB = Es - Os
        fvb = fv[b0:b0 + bpc].reshape(F, 2 * S)
        fvb[:, 0:2 * KD:2] = reA            # real, k = 0..255
        fvb[:, 1:2 * KD:2] = imA
        fvb[:, 514:1026:2] = reB[:, ::-1]   # real, k = 257..512
        fvb[:, 515:1027:2] = imB[:, ::-1]
        # col 256: even-s cos run is (-1)^(t+1), odd-s sin run is (-1)^t
        ue32 = uvp[b0:b0 + bpc, :, 0, :].reshape(F, M)
        vo32 = uvp[b0:b0 + bpc, :, 3, :].reshape(F, M)
        fvb[:, 512] = x0[:, 0] - ue32 @ alt
        fvb[:, 513] = -(vo32 @ alt)
        # Hermitian mirror: out[k] = conj(out[1024-k]) for k = 513..1023
        fvb[:, 1026::2] = fvb[:, 1022:0:-2]
        fvb[:, 1027::2] = -fvb[:, 1023:1:-2]
    return out


# revision 19
# speedup vs baseline: 1.4980x; 1.0303x over previous
"""Bass/Trainium2 kernel for nn_DFTLayer: out[b,f,k] = DFT_1024(x[b,f,:]).

reference: real = einsum('bfs,ks->bfk', x, wcos); imag = ... wsin
           out  = complex(real, -imag),  x: [16, 1024, 1024] f32.

Strategy (8 NeuronCores, data-parallel over batch, 2 batches/core):
  - Hermitian symmetry (x real): out[k] = conj(out[N-k]); device covers
    k = 0..255 directly and k = 257..512 via the radix-2 butterfly below;
    col 256 and the k = 513..1023 mirror are host-side.
  - Cosine/sine parity fold (host): u[s] = x[s] + x[N-s], v[s] = x[s] - x[N-s]
    over contraction slots s = 1..512 (u[512] = x[512], v[512] coeff is 0):
        real[k] = x[0] + sum_{s=1..512} u[s] cos(2*pi*k*s/N)
        imag[k] =        sum_{s=1..511} v[s] sin(2*pi*k*s/N)
  - Radix-2 split by parity of s (host): with ue[t] = u[2t+2], uo[t] = u[2t+1]
    (t = 0..255) and likewise ve/vo:
        E[k]  = ue @ wE[:,k],  O[k]  = uo @ wO[:,k]   (cos kernels)
        Es[k] = ve @ wEs[:,k], Os[k] = vo @ wOs[:,k]  (sin kernels)
        real[k]     = x[0] + E[k] + O[k]        k = 0..255
        real[512-k] = x[0] + E[k] - O[k]
        imag[k]     = Es[k] + Os[k],  imag[512-k] = -Es[k] + Os[k]
    This quarters the device matmul work vs the plain folded DFT.
  - Everything crossing HBM is bf16 (inputs pre-folded/transposed/cast on
    host, outputs cast bf16 on the way out): ~8.5 MB per core vs 18 MB for
    the f32 folded version; rel err ~3e-3, well under the 2e-2 gate.
  - Device program: w kernels stationary, moving operand is the transposed
    fold data in 512-wide streams; 64 matmuls, PSUM->SBUF bf16 casts spread
    over ACT/DVE/Pool, inputs on the sync HWDGE queue in consumption order
    (w on the scalar HWDGE queue in parallel), outputs split across the
    gpsimd SWDGE and scalar HWDGE queues. All butterflies/mirrors/
    corrections happen on the host.
"""

import sys

for _p in ("/opt/trn_rl_repo", "/root/.axon_site/_ro/trn_rl_repo"):
    if _p not in sys.path:
        sys.path.append(_p)

import numpy as np
import ml_dtypes
from contextlib import ExitStack

BF16 = np.dtype(ml_dtypes.bfloat16)

N_CORES = 8
B, F_FULL, S = 16, 1024, 1024          # x: [B, F_FULL, S]
F = (B // N_CORES) * F_FULL            # 2048 rows per core
M = 256                                # radix-2 contraction length
KD = 256                               # device freq cols per kernel (k = 0..255)
N_G = F // 512                         # 4 moving-operand groups of 512 rows
WARMUP_MM = 6                          # dummy matmuls to ramp the PE p-state

_CACHE = {}


def _build():
    """Build + compile the per-core Bass program (cached)."""
    if "nc" in _CACHE:
        return _CACHE["nc"]

    from concourse import bacc, tile, mybir

    f32 = mybir.dt.float32
    bf16 = mybir.dt.bfloat16

    nc = bacc.Bacc("TRN2", target_bir_lowering=False, debug=False)

    # uv row-block b = inp*2 + h (inp in ue,uo,ve,vo; h = row-half); within a
    # block: partition p, cols = tc*1024 + j for t = tc*128 + p, row h*1024+j
    uv_d = nc.dram_tensor("uv", [8 * 128, F], bf16, kind="ExternalInput")
    # w: partition p, cols = tc*1024 + (kern*2 + kc)*128 + q, k = kc*128 + q
    w_d = nc.dram_tensor("w", [128, 2 * 4 * KD], bf16, kind="ExternalInput")
    # eo rows: (kern*2 + kc)*128 + q  (freq k = kc*128 + q); col blocks
    # [half, g, 512] flatten to the 2048 core rows
    eo_d = nc.dram_tensor("eo", [8 * 128, 2, 2, 512], bf16, kind="ExternalOutput")

    with tile.TileContext(nc) as tc, ExitStack() as ctx:
        wpool = ctx.enter_context(tc.tile_pool(name="w", bufs=1))
        opool = ctx.enter_context(tc.tile_pool(name="o", bufs=4))
        ppool = ctx.enter_context(tc.tile_pool(name="p", bufs=4, space="PSUM"))

        f32r = mybir.dt.float32r

        # warm-up operand needs no DMA at all: a gpsimd memset right after
        # the template prologue (partial-partition-row DMAs proved ~4x
        # slower per byte than full 4 KB rows, so keep them off this path)
        wu_t = wpool.tile([128, 512], f32, tag="wu")
        nc.gpsimd.memset(wu_t[:], 1.0)

        # stationary DFT kernels: one full-row desc on the scalar HWDGE queue
        w_t = wpool.tile([128, 2, 1024], bf16, tag="w")
        nc.scalar.dma_start(w_t[:], w_d[:].rearrange("p (t j) -> p t j", t=2))

        # fold data on sync, consumption order, whole 4 KB-row blocks
        uv_ts = []
        for bidx in range(8):
            uv_t = wpool.tile([128, 2, 1024], bf16, tag=f"uv{bidx}")
            src = uv_d[bidx * 128:(bidx + 1) * 128, :]
            nc.sync.dma_start(uv_t[:], src.rearrange("p (t j) -> p t j", t=2))
            uv_ts.append(uv_t)

        # p-state warm-up: dummy matmuls ramp the PE clock (0.65 -> 2.4 GHz)
        # while the template boots and the first real operands stream in.
        # The tile comes from the main PSUM ring; its slot recycles after
        # the strictly-later real matmuls of the first reuse.
        ps_w = ppool.tile([128, 2, 512], f32, tag="ps")
        for i in range(WARMUP_MM):
            nc.tensor.matmul(ps_w[:, i % 2, :], wu_t[:, 0:128].bitcast(f32r),
                             wu_t[:].bitcast(f32r), start=True, stop=True)

        for kern in range(4):
            for kc in range(2):
                p = kern * 2 + kc
                last = p == 7
                out_t = opool.tile([128, 2, 2, 512], bf16)
                r0 = p * 128
                for half in range(2):
                    # half h covers g = 2h, 2h+1 -- both fed by uv block
                    # kern*2 + h; 2-bank PSUM tiles keep WAR hazards fine
                    ps = ppool.tile([128, 2, 512], f32, tag="ps")
                    src = uv_ts[kern * 2 + half]
                    for t in range(2):
                        lhsT = w_t[:, t, p * 128:(p + 1) * 128]
                        for g in range(2):
                            nc.tensor.matmul(
                                ps[:, g, :],
                                lhsT,
                                src[:, t, g * 512:(g + 1) * 512],
                                start=(t == 0),
                                stop=(t == 1),
                            )
                    # one combined 2-bank cast per half, alternating engines
                    # (cuts per-op overhead; only ACT/DVE can read PSUM)
                    if not last:
                        if (p * 2 + half) % 2 == 0:
                            nc.scalar.copy(out_t[:, half], ps[:])
                        else:
                            nc.vector.tensor_copy(out_t[:, half], ps[:])
                        nc.sync.dma_start(eo_d[r0:r0 + 128, half], out_t[:, half])
                    else:
                        # tail: split across both engines so the last PSUM
                        # banks drain in ~700ns; second desc on scalar
                        nc.scalar.copy(out_t[:, half, 0], ps[:, 0, :])
                        nc.vector.tensor_copy(out_t[:, half, 1], ps[:, 1, :])
                        for g in range(2):
                            eng = nc.sync if g == 0 else nc.scalar
                            eng.dma_start(eo_d[r0:r0 + 128, half, g],
                                          out_t[:, half, g])

    nc.compile()
    _CACHE["nc"] = nc
    return nc


def kernel(x, wsin, wcos):
    from concourse.bass_utils import run_bass_kernel_spmd

    x = np.asarray(x, dtype=np.float32)
    wsin = np.asarray(wsin, dtype=np.float32)
    wcos = np.asarray(wcos, dtype=np.float32)

    nc = _build()

    # radix-2 DFT kernels, sliced from the provided (symmetric) matrices:
    #   wE[t,k] = cos(2*pi*k*(2t+2)/N), wO[t,k] = cos(2*pi*k*(2t+1)/N)
    wE = wcos[2:513:2, 0:KD]
    wO = wcos[1:512:2, 0:KD]
    wEs = wsin[2:513:2, 0:KD]
    wOs = wsin[1:512:2, 0:KD]
    # [t, kern*256 + c] -> [tc, p, c] -> [p, tc*1024 + c]
    w_np = np.concatenate([wE, wO, wEs, wOs], axis=1).astype(BF16)
    w_np = np.ascontiguousarray(
        w_np.reshape(2, 128, 1024).transpose(1, 0, 2)).reshape(128, 2048)

    # host fold + parity split (f32), then bf16
    xa = x[:, :, 1:512]
    xb = x[:, :, 1023:512:-1]
    u = xa + xb                         # u[s], s = 1..511
    v = xa - xb
    uvp = np.empty((B, F_FULL, 4, M), dtype=np.float32)
    uvp[:, :, 0, :255] = u[:, :, 1::2]  # ue: s = 2,4,..,510
    uvp[:, :, 0, 255] = x[:, :, 512]    # ue[255] <- u[512] = x[512]
    uvp[:, :, 1, :] = u[:, :, 0::2]     # uo: s = 1,3,..,511
    uvp[:, :, 2, :255] = v[:, :, 1::2]  # ve
    uvp[:, :, 2, 255] = 0.0
    uvp[:, :, 3, :] = v[:, :, 0::2]     # vo
    uvp_bf = uvp.astype(BF16)

    bpc = B // N_CORES
    in_maps = []
    for c in range(N_CORES):
        blk = uvp_bf[c * bpc:(c + 1) * bpc].reshape(F, 4, M)
        # [row, i, t] -> [i, t, row] -> [i, tc, p, h, j] -> [i, h, p, tc, j]
        arr = blk.transpose(1, 2, 0).reshape(4, 2, 128, 2, 1024)
        uv_c = np.ascontiguousarray(arr.transpose(0, 3, 2, 1, 4)).reshape(8 * 128, F)
        in_maps.append({"uv": uv_c, "w": w_np})

    res = run_bass_kernel_spmd(
        nc, in_maps, core_ids=list(range(N_CORES)), **_CACHE.get("run_kwargs", {})
    )
    kernel.last_results = res

    # host assembly: butterflies, x[0] correction, col 256, Hermitian mirror
    alt = np.where(np.arange(M) % 2 == 0, np.float32(1.0), np.float32(-1.0))
    out = np.empty((B, F_FULL, S), dtype=np.complex64)
    fv = out.view(np.float32).reshape(B, F_FULL, 2 * S)
    for c in range(N_CORES):
        b0 = c * bpc
        eo = np.asarray(res.results[c]["eo"]).reshape(4, KD, F)
        E = eo[0].T.astype(np.float32)      # [F, KD]
        O = eo[1].T.astype(np.float32)
        Es = eo[2].T.astype(np.float32)
        Os = eo[3].T.astype(np.float32)
        x0 = x[b0:b0 + bpc, :, 0].reshape(F, 1)
        reA = E + O
        reA += x0
        reB = E - O
        reB += x0
        imA = Es + Os
        np.negative(imA, out=imA)           # out.imag = -imag_raw
        imB = Es - Os
        fvb = fv[b0:b0 + bpc].reshape(F, 2 * S)
        fvb[:, 0:2 * KD:2] = reA            # real, k = 0..255
        fvb[:, 1:2 * KD:2] = imA
        fvb[:, 514:1026:2] = reB[:, ::-1]   # real, k = 257..512
        fvb[:, 515:1027:2] = imB[:, ::-1]
        # col 256: even-s cos run is (-1)^(t+1), odd-s sin run is (-1)^t
        ue32 = uvp[b0:b0 + bpc, :, 0, :].reshape(F, M)
        vo32 = uvp[b0:b0 + bpc, :, 3, :].reshape(F, M)
        fvb[:, 512] = x0[:, 0] - ue32 @ alt
        fvb[:, 513] = -(vo32 @ alt)
        # Hermitian mirror: out[k] = conj(out[1024-k]) for k = 513..1023
        fvb[:, 1026::2] = fvb[:, 1022:0:-2]
        fvb[:, 1027::2] = -fvb[:, 1023:1:-2]
    return out
